# revision 34
# baseline (speedup 1.0000x reference)
"""Trainium2 Bass kernel (v11) for the CHNN constrained Hamiltonian vector field.

Math (unchanged): per sample,
    out = JdH - J DPhi X,   A X = DPhi^T JdH,   A = DPhi^T J DPhi
with chain constraints giving two 32x32 tridiagonal solves (G SPD):
    x1 = G^{-1} b0,  x0 = G^{-1}(K x1 - b1)
    b0 = Dr^T v, b1 = Ddot^T v - Dr^T Minv g, v = Minv p, g = dV/dr (MLP grad)
    out_r = v - Minv Dr x1 ; out_p = -g + Dr x0 + Ddot x1.

Deltas over the v3 baseline (trace-driven):
  - rt/W0 ship as one [65, 768] pack with a ones/b0 row: the L0 bias rides
    the matmul (K=65), one DMA issue fewer, and it is first on the Sync ring.
  - L2 backward via the sigmoid_and_others ACT table: dp2 = Sigmoid(ps2)
    replaces the Exp->Ln->Exp chain, sg1 = Sigmoid(ps1) replaces e1; one
    table swap hides under the L2 matmuls. Sign algebra lands on the same
    dp1 value as v3, so everything downstream is unchanged.
  - backward: gd computed directly on PE via the prepacked W0D =
    (W0^T Minv) D difference operator (wg3 fetch range fixed to actually
    cover it); the gt/gd DVE+Pool chain is gone.
  - PCR a-path: tq = fC*rt is computed first and reused (uw = tq*fC, fp16),
    one fewer [128,288] DVE mul per level.
  - pair-solve off-critical pieces (aswap/feff/ffq/m2x/alf/phf) on Pool.
  - out_p stored/DMA'd in four chunks alternating Sync/Scalar rings.
  - PE warmup: 6 back-to-back 512-col dummy matmuls during the input-DMA
    window (no mid-kernel keepalives; they only queued ahead of real work).

Sharding: pure data-parallel over batch across 8 cores.
"""

import numpy as np

N, D = 32, 2
ZD = 128
BS = 4096
NCORES = 8
BSL = BS // NCORES   # 512
C4 = BSL // 128      # 4 batch chunks
PAD = 16
W, TW = 128, 160
NWARM = 5            # PE warmup matmuls (512-col, back-to-back)

# fp16 weight block column offsets (OW0D must sit inside the wg3 fetch
# range [OW2T, OW2T+1280) — the gdm matmul reads it)
OW0, OW1, OW2, OW2T, OW1T, OW0T, OW0D, OBR, WTOT = 0, 256, 768, 1280, 1792, 2304, 2432, 2560, 3584
# fp32 misc block: jnv(128) | fp16 misc block: invm(256) fco(128) eco(128)
OJN, C32 = 0, 128
OIV, OFC, OEC, C16 = 0, 256, 384, 512


def build_program(debug=False):
    from contextlib import ExitStack

    import concourse.bass as bass
    import concourse.mybir as mybir
    import concourse.tile as tile
    from concourse import bacc
    from concourse.hw_specs import get_activation_tables
    import bass_rust as _bass_rust

    f32 = mybir.dt.float32
    f16 = mybir.dt.float16
    AF = mybir.ActivationFunctionType
    OP = mybir.AluOpType

    class PinnedActBacc(bacc.Bacc):
        # Exp/Ln live on natural_log_exp_and_others; Sigmoid on
        # sigmoid_and_others (real 800-bucket table). Emptying all other
        # tables preserves act_func_set_id indexing; exactly one swap is
        # emitted (before the first Sigmoid, after the last Ln).
        def insert_act_table_loads(self):
            has_activation = any(
                isinstance(i, mybir.InstActivation)
                for b in self.main_func.blocks
                for i in b.instructions
            )
            if not has_activation:
                return
            keep = ("natural_log_exp_and_others", "sigmoid_and_others")
            tables = [
                (name, funcs if name in keep else set())
                for name, funcs in get_activation_tables(self.m.arch).items()
            ]
            _bass_rust.insert_act_table_loads(self, tables)

    nc = PinnedActBacc(
        "TRN2",
        target_bir_lowering=False,
        debug=debug,
        enable_asserts=True,
        num_devices=NCORES,
    )

    r16d = nc.dram_tensor("r16d", [128, 256], f16, kind="ExternalInput")
    v16d = nc.dram_tensor("v16d", [128, 256], f16, kind="ExternalInput")
    rtw0d = nc.dram_tensor("rtw0d", [65, 768], f16, kind="ExternalInput")
    wpk = nc.dram_tensor("wpk", [128, WTOT], f16, kind="ExternalInput")
    cp32 = nc.dram_tensor("cp32", [128, C32], f32, kind="ExternalInput")
    cp16 = nc.dram_tensor("cp16", [128, C16], f16, kind="ExternalInput")
    out = nc.dram_tensor("out", [BSL, ZD], f32, kind="ExternalOutput")

    with tile.TileContext(nc) as tc:
        with ExitStack() as ctx:
            const = ctx.enter_context(tc.tile_pool(name="const", bufs=1))
            main = ctx.enter_context(tc.tile_pool(name="main", bufs=1))
            scr = ctx.enter_context(tc.tile_pool(name="scr", bufs=2))
            psmm = ctx.enter_context(tc.tile_pool(name="psmm", bufs=3, space="PSUM"))
            pswm = ctx.enter_context(tc.tile_pool(name="pswm", bufs=1, space="PSUM"))

            # ---- PE warmup fodder + input DMAs (critical first per ring) ----
            wmup = const.tile([128, BSL], f16)
            nc.gpsimd.memset(wmup, 0.0)
            ones = const.tile([1, BSL], f16)
            nc.gpsimd.memset(ones, 1.0)

            rtw0 = const.tile([65, 768], f16)
            nc.sync.dma_start(out=rtw0, in_=rtw0d.ap())
            rT = rtw0[:, 0:512]
            wg0 = rtw0[:, 512:768]
            r16 = main.tile([128, 256], f16)
            nc.scalar.dma_start(out=r16, in_=r16d.ap())
            v16 = main.tile([128, 256], f16)
            nc.scalar.dma_start(out=v16, in_=v16d.ap())
            c16 = const.tile([128, C16], f16)
            nc.scalar.dma_start(out=c16, in_=cp16.ap())
            brow = const.tile([1, 1024], f16)
            nc.sync.dma_start(out=brow, in_=wpk.ap()[0:1, OBR : OBR + 1024])
            c32 = const.tile([128, C32], f32)
            nc.sync.dma_start(out=c32, in_=cp32.ap())
            wg12 = const.tile([128, 1024], f16)
            nc.sync.dma_start(out=wg12, in_=wpk.ap()[:, OW1 : OW1 + 1024])
            wg3 = const.tile([128, 1280], f16)
            nc.sync.dma_start(out=wg3, in_=wpk.ap()[:, OW2T : OW2T + 1280])

            pwarm = pswm.tile([128, BSL], f32, tag="wm", name="pwarm")
            for _ in range(NWARM):
                nc.tensor.matmul(pwarm, wmup[:, 0:128], wmup, start=True, stop=True)

            w0sb = wg0
            w1sb = wg12[:, 0:512].rearrange("p (k n) -> p k n", k=2)
            w2sb = wg12[:, 512:1024].rearrange("p (k n) -> p k n", k=2)
            w2tsb = wg3[:, 0:512].rearrange("p (k n) -> p k n", k=2)
            w1tsb = wg3[:, 512:1024].rearrange("p (k n) -> p k n", k=2)
            w0tsb = wg3[:, 1024:1152].rearrange("p (k n) -> p k n", k=2)
            w0dsb = wg3[:, 1152:1280].rearrange("p (k n) -> p k n", k=2)
            jnvP = c32[:, OJN : OJN + 128]
            invmP = c16[:, OIV : OIV + 256]
            fcoP = c16[:, OFC : OFC + 128]
            ecoP = c16[:, OEC : OEC + 128]

            def bmm(ps, m, off):  # K=1 ones-row bias matmul opening a group
                nc.tensor.matmul(
                    ps[:, m, :], brow[0:1, off + m * 128 : off + (m + 1) * 128],
                    ones, start=True, stop=False,
                )

            # ---- PCR + chain tiles; memsets spread across idle engines ----
            apcr = main.tile([128, TW], f32)
            nc.gpsimd.memset(apcr, 1.0)
            ft = [main.tile([128, TW], f16, tag=f"ft{i}", name=f"ft{i}") for i in range(2)]
            bt = [main.tile([128, TW], f32, tag=f"bt{i}", name=f"bt{i}") for i in range(2)]
            b16 = [main.tile([128, TW], f16, tag=f"b16{i}", name=f"b16{i}") for i in range(2)]
            for tl_ in ft:
                nc.vector.memset(tl_, 0.0)
            for tl_ in bt:
                nc.gpsimd.memset(tl_, 0.0)
            for tl_ in b16:
                nc.vector.memset(tl_, 0.0)
            kkp = main.tile([128, TW], f16)
            nc.gpsimd.memset(kkp, 0.0)
            x1p = main.tile([128, TW], f16)
            nc.vector.memset(x1p, 0.0)

            # ================= MLP L0 (bias via K=65 ones row) =================
            ps0 = psmm.tile([128, 2, BSL], f32, tag="mm", name="ps0")
            for m in range(2):
                nc.tensor.matmul(
                    ps0[:, m, :], w0sb[:, m * 128 : (m + 1) * 128], rT,
                    start=True, stop=True,
                )
            h0 = main.tile([128, 2, BSL], f16)
            tE0 = scr.tile([128, 2, BSL], f16, tag="tE", name="tE0")
            for m in range(2):
                nc.scalar.activation(tE0[:, m, :], ps0[:, m, :], AF.Exp)
                nc.scalar.activation(h0[:, m, :], tE0[:, m, :], AF.Ln, bias=1.0)

            # ---- chain quantities (batch-major, fp16) ----
            def c3(t, f=64):
                return t.rearrange("p (c f) -> p c f", f=f)

            ut = main.tile([128, 256], f16)
            nc.vector.tensor_sub(
                c3(ut)[:, :, 2:64], c3(r16)[:, :, 0:62], c3(r16)[:, :, 2:64]
            )
            nc.gpsimd.tensor_copy(c3(ut)[:, :, 0:2], c3(r16)[:, :, 0:2])
            wt = main.tile([128, 256], f16)
            nc.vector.tensor_sub(
                c3(wt)[:, :, 2:64], c3(v16)[:, :, 0:62], c3(v16)[:, :, 2:64]
            )
            nc.gpsimd.tensor_copy(c3(wt)[:, :, 0:2], c3(v16)[:, :, 0:2])

            def prodpair(dst, xa, ya, npair, eng=None):
                e = eng or nc.vector
                pr = scr.tile([128, 256], f16, tag="prod", name="pr")
                if npair == 32:
                    e.tensor_mul(pr, xa, ya)
                else:
                    e.tensor_mul(c3(pr)[:, :, 0:62], xa, ya)
                pe = pr.rearrange("p (c i d) -> p c i d", d=2, i=32)
                d3 = dst.rearrange("p (c i) -> p c i", i=32)
                e.tensor_add(
                    d3[:, :, 0:npair], pe[:, :, 0:npair, 0], pe[:, :, 0:npair, 1]
                )

            dcc = main.tile([128, 128], f16)
            prodpair(dcc, ut, ut, 32)
            nc.gpsimd.tensor_mul(apcr[:, PAD : PAD + 128], jnvP, dcc)
            cuu = main.tile([128, 128], f16)
            nc.gpsimd.memset(cuu, 0.0)
            prodpair(cuu, c3(ut)[:, :, 0:62], c3(ut)[:, :, 2:64], 31)
            nc.gpsimd.tensor_mul(ft[0][:, PAD : PAD + 128], fcoP, cuu)
            prodpair(bt[0][:, PAD : PAD + 128], ut, wt, 32, nc.gpsimd)
            pww = main.tile([128, 128], f16)
            prodpair(pww, wt, wt, 32, nc.gpsimd)
            cwu = main.tile([128, 128], f16)
            nc.gpsimd.memset(cwu, 0.0)
            prodpair(cwu, c3(wt)[:, :, 0:62], c3(ut)[:, :, 2:64], 31, nc.gpsimd)
            cuw = main.tile([128, 128], f16)
            nc.gpsimd.memset(cuw, 0.0)
            prodpair(cuw, c3(ut)[:, :, 0:62], c3(wt)[:, :, 2:64], 31, nc.gpsimd)
            cdt = scr.tile([128, 128], f16, tag="cdt", name="cdt")
            nc.gpsimd.tensor_sub(cdt, cwu, cuw)
            nc.gpsimd.tensor_mul(kkp[:, PAD : PAD + 128], ecoP, cdt)

            # ---- PCR machinery ----
            def A_(t, off=0, w=W):
                return t[:, PAD + off : PAD + off + w]

            rt_ = main.tile([128, TW], f32)
            tl = [main.tile([128, 2, 144], f16, tag=f"tl{i}", name=f"tl{i}")
                  for i in range(5)]
            shifts = [1, 2, 4, 8, 16]

            def P2(tile_, off, ostride, w):
                b = tile_
                return bass.AP(
                    tensor=b.tensor,
                    offset=b.offset + PAD + off,
                    ap=[b.ap[0], [ostride, 2], [1, w]],
                )

            def pcr_level(lev, cur):
                # a-chain on DVE (uw = tq*fC, one mul fewer than v3);
                # b-path on Pool
                s = shifts[lev]
                W2S = W + s
                fC, fN = ft[cur], ft[1 - cur]
                bC, bN = bt[cur], bt[1 - cur]
                nc.vector.reciprocal_approx_fast(
                    A_(rt_, -s, W + 2 * s), A_(apcr, -s, W + 2 * s)
                )
                tq = tl[lev]
                nc.vector.tensor_mul(
                    tq[:, :, 0:W2S], P2(fC, -s, 0, W2S), P2(rt_, -s, s, W2S)
                )
                uw = scr.tile([128, 2, 144], f16, tag="uw", name="uw")
                nc.vector.tensor_mul(
                    uw[:, :, 0:W2S], tq[:, :, 0:W2S], P2(fC, -s, 0, W2S)
                )
                nc.vector.tensor_sub(A_(apcr), A_(apcr), uw[:, 0, 0:W])
                nc.vector.tensor_sub(A_(apcr), A_(apcr), uw[:, 1, s : s + W])
                nc.vector.tensor_mul(A_(fN), tq[:, 1, s : s + W], A_(fC, s))
                tm = scr.tile([128, 2, 144], f32, tag="tm", name="tm")
                nc.gpsimd.tensor_mul(
                    tm[:, :, 0:W2S], tq[:, :, 0:W2S], P2(bC, -s, s, W2S)
                )
                nc.gpsimd.tensor_add(A_(bN), A_(bC), tm[:, 0, 0:W])
                nc.gpsimd.tensor_add(A_(bN), A_(bN), tm[:, 1, s : s + W])

            # ================= MLP L1 =================
            ps1 = psmm.tile([128, 2, BSL], f32, tag="mm", name="ps1")
            for m in range(2):
                bmm(ps1, m, 256)
            for k in range(2):
                for m in range(2):
                    nc.tensor.matmul(
                        ps1[:, m, :], w1sb[:, k, m * 128 : (m + 1) * 128],
                        h0[:, k, :], start=False, stop=(k == 1),
                    )
            h1 = main.tile([128, 2, BSL], f16)
            tE1 = scr.tile([128, 2, BSL], f16, tag="tE", name="tE1")
            for m in range(2):
                nc.scalar.activation(tE1[:, m, :], ps1[:, m, :], AF.Exp)
                nc.scalar.activation(h1[:, m, :], tE1[:, m, :], AF.Ln, bias=1.0)
            e0 = main.tile([128, 2, BSL], f16)
            nc.scalar.activation(e0, h0, AF.Exp, scale=-1.0)

            pcr_level(0, 0)
            pcr_level(1, 1)
            pcr_level(2, 0)

            # ================= MLP L2 =================
            ps2 = psmm.tile([128, 2, BSL], f32, tag="mm", name="ps2")
            for m in range(2):
                bmm(ps2, m, 512)
            for k in range(2):
                for m in range(2):
                    nc.tensor.matmul(
                        ps2[:, m, :], w2sb[:, k, m * 128 : (m + 1) * 128],
                        h1[:, k, :], start=False, stop=(k == 1),
                    )
            # sg2 = Sigmoid(ps2) directly (table swap: sigmoid_and_others).
            # ps3 = W2W^T-matvec of sg2 = +u1 = -(v3's ps3); dp1 = sg1*ps3
            # lands on the SAME value as v3's dp1, so everything downstream
            # is unchanged.
            sg2 = main.tile([128, 2, BSL], f16)
            for m in range(2):
                nc.scalar.activation(sg2[:, m, :], ps2[:, m, :], AF.Sigmoid)
            sg1 = main.tile([128, 2, BSL], f16)
            for m in range(2):
                nc.scalar.activation(sg1[:, m, :], ps1[:, m, :], AF.Sigmoid)

            # ---- PCR level 3 + stride-16 pair-solve (fills the DVE/Pool
            # window while PE runs the backward matmuls) ----
            pcr_level(3, 1)

            def swap16(t):
                # partner read: col j -> col j XOR 16 within each 32-chunk
                return bass.AP(
                    tensor=t.tensor,
                    offset=t.offset + PAD + 16,
                    ap=[t.ap[0], [32, 4], [-16, 2], [1, 16]],
                )

            aswap = main.tile([128, W], f32)
            nc.gpsimd.tensor_copy(aswap, swap16(apcr))
            feff = main.tile([128, W], f16)
            nc.gpsimd.tensor_add(feff, A_(ft[0]), A_(ft[0], -16))
            ffq = scr.tile([128, W], f32, tag="ffq", name="ffq")
            nc.gpsimd.tensor_mul(ffq, feff, feff)
            det = main.tile([128, W], f32)
            nc.vector.tensor_mul(det, A_(apcr), aswap)
            nc.vector.tensor_sub(det, det, ffq)
            rdet = main.tile([128, W], f32)
            nc.vector.reciprocal_approx_fast(rdet, det)

            # x1 = (aswap*b + feff*b_partner) * rdet; the muls don't need
            # rdet so they overlap the det/recip chain
            m1x = scr.tile([128, W], f32, tag="m1x", name="m1x")
            nc.vector.tensor_mul(m1x, aswap, A_(bt[0]))
            m2x = scr.tile([128, W], f32, tag="m2x", name="m2x")
            nc.gpsimd.tensor_mul(m2x, feff, swap16(bt[0]))
            nc.vector.tensor_add(m1x, m1x, m2x)
            nc.vector.tensor_mul(A_(x1p), m1x, rdet)

            alf = main.tile([128, W], f16)
            nc.gpsimd.tensor_mul(alf, aswap, rdet)
            phf = main.tile([128, W], f16)
            nc.gpsimd.tensor_mul(phf, feff, rdet)

            # ---- x1 outputs (out_r) + early rhs2 prep ----
            outt = main.tile([128, C4, ZD], f32)
            outv = out.ap().rearrange("(p j) f -> p j f", j=4)
            def bcast2(t):
                # [128, (c i)] -> [128, (c i), d=2 stride-0] broadcast read
                return bass.AP(
                    tensor=t.tensor,
                    offset=t.offset + PAD,
                    ap=[t.ap[0], [1, 128], [0, 2]],
                )

            def drx(dst3, src3, eng):
                eng.tensor_add(
                    dst3[:, :, 0:2], src3[:, :, 2:4], src3[:, :, 0:2]
                )
                eng.tensor_sub(
                    dst3[:, :, 2:62], src3[:, :, 4:64], src3[:, :, 2:62]
                )
                eng.tensor_scalar_mul(
                    dst3[:, :, 62:64], src3[:, :, 62:64], -1.0
                )

            A1 = main.tile([128, 256], f16)
            nc.vector.scalar_tensor_tensor(
                A1.rearrange("p (ci d) -> p ci d", d=2), ut.rearrange("p (ci d) -> p ci d", d=2), 4.0, bcast2(x1p),
                op0=OP.mult, op1=OP.mult,
            )
            drA = main.tile([128, 256], f16)
            drx(c3(drA), c3(A1), nc.gpsimd)
            sD = scr.tile([128, 256], f16, tag="sD", name="sD")
            nc.gpsimd.tensor_mul(sD, drA, invmP)
            nc.gpsimd.tensor_sub(outt[:, :, 0:64], c3(v16), c3(sD))
            nc.sync.dma_start(out=outv[:, :, 0:64], in_=outt[:, :, 0:64])
            bs_ = main.tile([128, 256], f16)
            nc.vector.scalar_tensor_tensor(
                bs_.rearrange("p (ci d) -> p ci d", d=2), wt.rearrange("p (ci d) -> p ci d", d=2), 4.0, bcast2(x1p),
                op0=OP.mult, op1=OP.mult,
            )

            # pre2 = K x1 - pww (all early; only -pugd needs g)
            t1 = scr.tile([128, 128], f16, tag="t1", name="t1")
            nc.gpsimd.tensor_mul(t1, A_(kkp), A_(x1p, 1))
            t2k = scr.tile([128, 128], f16, tag="t2k", name="t2k")
            nc.gpsimd.tensor_mul(t2k, A_(kkp, -1), A_(x1p, -1))
            ttk = scr.tile([128, 128], f16, tag="ttk", name="ttk")
            nc.gpsimd.tensor_sub(ttk, t1, t2k)
            pre2 = main.tile([128, 128], f16)
            nc.gpsimd.tensor_sub(pre2, ttk, pww)

            # ================= MLP backward =================
            # Emission order puts the chain-critical STTs and matmuls AHEAD
            # of the PCR/out_r filler so the scheduler never starves them.
            ps3 = psmm.tile([128, 2, BSL], f32, tag="mm", name="ps3")
            for k in range(2):
                for m in range(2):
                    nc.tensor.matmul(
                        ps3[:, m, :], w2tsb[:, k, m * 128 : (m + 1) * 128],
                        sg2[:, k, :], start=(k == 0), stop=(k == 1),
                    )
            # dp1 = sg1 * ps3 = +dp1_true (matches v3's dp1 value)
            dp1 = main.tile([128, 2, BSL], f16)
            with tc.high_priority():
                for m in range(2):
                    nc.vector.scalar_tensor_tensor(
                        dp1[:, m, :], sg1[:, m, :], 1.0, ps3[:, m, :],
                        op0=OP.mult, op1=OP.mult,
                    )

            ps4 = psmm.tile([128, 2, BSL], f32, tag="mm", name="ps4")
            for k in range(2):
                for m in range(2):
                    nc.tensor.matmul(
                        ps4[:, m, :], w1tsb[:, k, m * 128 : (m + 1) * 128],
                        dp1[:, k, :], start=(k == 0), stop=(k == 1),
                    )
            dp0 = main.tile([128, 2, BSL], f16)
            with tc.high_priority():
                for m in range(2):
                    nc.vector.scalar_tensor_tensor(
                        dp0[:, m, :], e0[:, m, :], 1.0, ps4[:, m, :],
                        op0=OP.subtract, op1=OP.mult,
                    )

            # gdm = dp0 @ W0D = D^T(-Minv g) directly on PE (chunk-major)
            gdm = psmm.tile([128, C4, 64], f32, tag="mm", name="gdm")
            for c in range(C4):
                for k in range(2):
                    nc.tensor.matmul(
                        gdm[:, c, :], dp0[:, k, c * 128 : (c + 1) * 128],
                        w0dsb[:, k, :], start=(k == 0), stop=(k == 1),
                    )
            gbm3 = psmm.tile([128, C4, 64], f32, tag="mm", name="gbm3")
            for c in range(C4):
                for k in range(2):
                    nc.tensor.matmul(
                        gbm3[:, c, :], dp0[:, k, c * 128 : (c + 1) * 128],
                        w0tsb[:, k, :], start=(k == 0), stop=(k == 1),
                    )

            # ---- tail: pugd from gdm (PE did Minv+D), then fp16 replay ----
            pugd = main.tile([128, 128], f16)
            with tc.high_priority():
                prg = scr.tile([128, 256], f16, tag="prg", name="prg")
                nc.vector.tensor_mul(c3(prg), c3(ut), gdm)
                prge = prg.rearrange("p (c i d) -> p c i d", d=2, i=32)
                pugd3 = pugd.rearrange("p (c i) -> p c i", i=32)
                nc.vector.tensor_add(pugd3, prge[:, :, :, 0], prge[:, :, :, 1])
                nc.vector.tensor_sub(A_(b16[0]), pre2, pugd)

            cur = 0
            for lev in range(4):
                s = shifts[lev]
                W2S = W + s
                bC, bN = b16[cur], b16[1 - cur]
                tm = scr.tile([128, 2, 144], f16, tag="tm16", name="tm16")
                nc.vector.tensor_mul(
                    tm[:, :, 0:W2S], tl[lev][:, :, 0:W2S], P2(bC, -s, s, W2S)
                )
                nc.vector.tensor_add(A_(bN), A_(bC), tm[:, 0, 0:W])
                nc.vector.tensor_add(A_(bN), A_(bN), tm[:, 1, s : s + W])
                cur = 1 - cur

            # x0 = alf*b + phf*b_partner (pair-solve apply); only
            # [PAD, PAD+W) is ever read (bcast2), so no pad memset needed
            x0t = main.tile([128, TW], f16)
            nc.vector.tensor_mul(A_(x0t), alf, A_(b16[0]))
            m2y = scr.tile([128, W], f16, tag="m2x", name="m2y")
            nc.vector.tensor_mul(m2y, phf, swap16(b16[0]))
            nc.vector.tensor_add(A_(x0t), A_(x0t), m2y)

            Bt1 = main.tile([128, 256], f16)
            nc.vector.scalar_tensor_tensor(
                Bt1.rearrange("p (ci d) -> p ci d", d=2), ut.rearrange("p (ci d) -> p ci d", d=2), 4.0, bcast2(x0t),
                op0=OP.mult, op1=OP.mult,
            )
            nc.vector.tensor_add(Bt1, Bt1, bs_)
            drB = main.tile([128, 256], f16)
            drx(c3(drB), c3(Bt1), nc.vector)
            for c in range(C4):
                nc.vector.tensor_add(
                    outt[:, c : c + 1, 64:128], c3(drB)[:, c : c + 1, :],
                    gbm3[:, c : c + 1, :],
                )
                ring = nc.sync if c % 2 == 0 else nc.scalar
                ring.dma_start(
                    out=outv[:, c : c + 1, 64:128], in_=outt[:, c : c + 1, 64:128]
                )

    nc.compile()
    return nc


def host_inputs(inputs):
    """Host-side prep: per-core input maps (weights replicated, z sharded)."""
    f = lambda x: np.ascontiguousarray(np.asarray(x, np.float32))
    z = f(inputs["z"])
    W0, W1, W2, W3 = f(inputs["W0"]), f(inputs["W1"]), f(inputs["W2"]), f(inputs["W3"])

    wpk = np.zeros((128, WTOT), np.float16)
    W2W = np.ascontiguousarray(W2 * W3[:, 0][None, :])
    for k in range(2):
        sl = slice(k * 128, (k + 1) * 128)
        wpk[:, OW1 + 256 * k : OW1 + 256 * (k + 1)] = W1[sl]
        wpk[:, OW2 + 256 * k : OW2 + 256 * (k + 1)] = W2[sl]
        wpk[:, OW2T + 256 * k : OW2T + 256 * (k + 1)] = W2W.T[sl]
        wpk[:, OW1T + 256 * k : OW1T + 256 * (k + 1)] = W1.T[sl]
        wpk[:, OW0T + 64 * k : OW0T + 64 * (k + 1)] = W0.T[sl]
    inv_h = np.exp(-f(inputs["m_params"])[:, 0])
    invm64_h = np.repeat(inv_h, 2)
    W0TS = W0.T * invm64_h[None, :]
    DD = np.zeros((64, 64), np.float32)
    for c in range(32):
        for dd in range(2):
            col = 2 * c + dd
            if c == 0:
                DD[col, col] = 1.0
            else:
                DD[2 * (c - 1) + dd, col] = 1.0
                DD[col, col] = -1.0
    W0TD = W0TS @ DD
    for k in range(2):
        sl = slice(k * 128, (k + 1) * 128)
        wpk[:, OW0D + 64 * k : OW0D + 64 * (k + 1)] = W0TD[sl]
    wpk[0, OBR : OBR + 256] = f(inputs["b0"])
    wpk[0, OBR + 256 : OBR + 512] = f(inputs["b1"])
    wpk[0, OBR + 512 : OBR + 768] = f(inputs["b2"])
    wpk[0, OBR + 768 : OBR + 1024] = -W2W.T.sum(0)   # bwd2 "+1" fold

    c32row = np.zeros((128, C32), np.float32)
    inv = np.exp(-f(inputs["m_params"])[:, 0])
    jnv = np.empty(32, np.float32)
    jnv[0] = inv[0]
    jnv[1:] = inv[:-1] + inv[1:]
    c32row[:, OJN : OJN + 128] = np.tile(4.0 * jnv, 4)[None, :]
    c32 = np.ascontiguousarray(c32row)

    invm64 = np.repeat(inv, 2)
    eco = (-4.0 * inv).astype(np.float32)
    eco[0] = 4.0 * inv[0]
    eco[31] = 0.0
    c16row = np.zeros(C16, np.float32)
    c16row[OIV : OIV + 256] = np.tile(invm64, 4)
    c16row[OFC : OFC + 128] = np.tile(-eco, 4)
    c16row[OEC : OEC + 128] = np.tile(eco, 4)
    c16 = np.broadcast_to(c16row.astype(np.float16), (128, C16)).copy()

    shared = {"wpk": wpk, "cp32": c32, "cp16": np.ascontiguousarray(c16)}
    maps = []
    for i in range(NCORES):
        zi = np.ascontiguousarray(z[i * BSL : (i + 1) * BSL])
        # rT16[f, c*128 + p] = z[4p + c, f]  (matches batch permutation);
        # row 64 = ones / b0 for the K=65 bias fold
        rtw0 = np.ones((65, 768), np.float16)
        rtw0[0:64, 0:512] = np.ascontiguousarray(
            zi[:, 0:64].reshape(128, 4, 64).transpose(2, 1, 0).reshape(64, BSL)
        ).astype(np.float16)
        rtw0[0:64, 512:768] = W0.astype(np.float16)
        rtw0[64, 512:768] = f(inputs["b0"]).astype(np.float16)
        r16pk = np.ascontiguousarray(
            zi[:, 0:64].reshape(128, 4, 64).reshape(128, 256)
        ).astype(np.float16)
        v16pk = np.ascontiguousarray(
            (zi[:, 64:128] * invm64[None, :]).reshape(128, 4, 64).reshape(128, 256)
        ).astype(np.float16)
        maps.append({**shared, "r16d": r16pk, "v16d": v16pk, "rtw0d": rtw0})
    return maps


TRACE = False
TMPDIR = None
LAST_RESULT = None


def kernel(**inputs) -> np.ndarray:
    global LAST_RESULT
    from concourse.bass_utils import run_bass_kernel_spmd

    nc = build_program()
    in_maps = host_inputs(inputs)
    res = run_bass_kernel_spmd(
        nc, in_maps, list(range(NCORES)), trace=TRACE, tmpdir=TMPDIR
    )
    LAST_RESULT = res
    return np.concatenate([res.results[i]["out"] for i in range(NCORES)], axis=0)
